# revision 17
# baseline (speedup 1.0000x reference)
"""LocalitySelfAttention TRN2 kernel.

B=4, N=2048, C=768, H=12, D=64.  8 cores: core c -> batch c//2, heads
6*(c%2) .. 6*(c%2)+6 (6 contiguous heads).  Each core computes its heads'
qkv projection, attention (scores kept transposed: [keys, qrows] so softmax
sums come from a fused ones-column in the AV matmul), and a partial output
projection restricted to its heads' 384 rows of w_proj.  Host sums the two
partials per batch and adds b_proj.

All-transposed dataflow: host passes x[b].T; q/k are produced transposed
([64, 2048] per head, stationary = w_qkv columns), v natural ([2048, 64],
stationary = xT blocks).  ST block = kT_blk.T @ qT -> [128 keys, qrows];
exp on ACT with scale=0.125; AV: lhsT = v_aug [keys, 64+1(ones)], rhs = PT
-> outT_aug [65, qrows] accumulated over key blocks; row 64 = softmax sums.
Diagonal temperature factor: one [128,128] mask multiply per (head, kblock)
on the diagonal sub-block before exp.
"""

import sys
import numpy as np

if "/opt/trn_rl_repo" not in sys.path:
    sys.path.insert(0, "/opt/trn_rl_repo")

B, N, C, H = 4, 2048, 768, 12
D = C // H          # 64
NH = 6              # heads per core
P = 128
CT = C // P         # 6 contraction tiles
KB = N // P         # 16 key blocks
QC = N // 512       # 4 free-dim chunks of 512
SCALE = float(D) ** -0.5  # 0.125

_CACHE = {}


def _build_program():
    import concourse.bass as bass
    import concourse.mybir as mybir
    import concourse.tile as tile
    from concourse import bacc
    from concourse.masks import make_identity

    f32 = mybir.dt.float32
    Exp = mybir.ActivationFunctionType.Exp
    mult = mybir.AluOpType.mult
    add = mybir.AluOpType.add

    nc = bacc.Bacc()
    xT = nc.dram_tensor("xT", [C, N], f32, kind="ExternalInput")
    wqkv = nc.dram_tensor("wqkv", [C, 3 * NH * D], f32, kind="ExternalInput")
    wproj = nc.dram_tensor("wproj", [NH * D, C], f32, kind="ExternalInput")
    temp = nc.dram_tensor("temp", [P, NH], f32, kind="ExternalInput")
    outT = nc.dram_tensor("outT", [C, N], f32, kind="ExternalOutput")
    rdram = nc.dram_tensor("rscratch", [NH, N], f32)  # internal: recip rows

    with tile.TileContext(nc) as tc:
        with (
            tc.tile_pool(name="const", bufs=1) as constp,
            tc.tile_pool(name="persist", bufs=1) as persist,
            tc.tile_pool(name="psum", bufs=2, space=bass.MemorySpace.PSUM) as psum,
        ):
            # ---- setup: identity, temperature masks -------------------
            ident = constp.tile([P, P], f32, tag="ident")
            make_identity(nc, ident[:])
            tbc = constp.tile([P, NH], f32, tag="tbc")
            nc.sync.dma_start(tbc[:, :], temp[:, :])
            ntb = constp.tile([P, NH], f32, tag="ntb")
            nc.vector.tensor_scalar_mul(ntb[:, :], tbc[:, :], -1.0)
            masks = constp.tile([P, NH, P], f32, tag="masks")
            for h in range(NH):
                # mask_h = 1 - t_h * I
                nc.vector.tensor_scalar(
                    masks[:, h, :], ident[:], ntb[:, h : h + 1], 1.0, mult, add
                )

            # persistent across qkv+attention: qT/kT pairs, v_aug
            qkT = persist.tile([P, 2 * NH, N], f32, tag="qkT")  # idx 0-2 q, 3-5 k
            vaug = persist.tile([P, KB, NH, D + 1], f32, tag="vaug")
            nc.vector.memset(vaug[:, :, :, D : D + 1], 1.0)

            # ---- phase 1: qkv projection ------------------------------
            with tc.tile_pool(name="qin", bufs=1) as qin:
                xt = qin.tile([P, CT, N], f32, tag="xt")
                for t in range(CT):
                    nc.sync.dma_start(xt[:, t, :], xT[t * P : (t + 1) * P, :])
                wq = qin.tile([P, CT, 3 * NH * D], f32, tag="wq")
                for t in range(CT):
                    nc.sync.dma_start(wq[:, t, :], wqkv[t * P : (t + 1) * P, :])

                # q,k transposed: 6 groups of 128 cols (3 q head-pairs, 3 k)
                for g in range(6):
                    ps = psum.tile([P, N], f32, tag="ps")
                    for t in range(CT):
                        for qc in range(QC):
                            nc.tensor.matmul(
                                ps[:, qc * 512 : (qc + 1) * 512],
                                wq[:, t, g * P : (g + 1) * P],
                                xt[:, t, qc * 512 : (qc + 1) * 512],
                                start=(t == 0),
                                stop=(t == CT - 1),
                            )
                    nc.vector.tensor_copy(qkT[:, g, :], ps[:])

                # v natural, interleaved with ones column
                for rb_i in range(KB):
                    psv = psum.tile([P, NH * D], f32, tag="ps")
                    for t in range(CT):
                        nc.tensor.matmul(
                            psv[:],
                            xt[:, t, rb_i * P : (rb_i + 1) * P],
                            wq[:, t, 2 * NH * D : 3 * NH * D],
                            start=(t == 0),
                            stop=(t == CT - 1),
                        )
                    nc.vector.tensor_copy(
                        vaug[:, rb_i, :, 0:D],
                        psv[:].rearrange("p (h d) -> p h d", h=NH),
                    )

            # ---- phases 2+3 (workspace reuses qkv staging space) ------
            with (
                tc.tile_pool(name="ph2", bufs=1) as ph2,
                tc.tile_pool(name="pt", bufs=3) as ptp,
                tc.tile_pool(name="rb", bufs=2) as rbp,
            ):
                attnT = ph2.tile([P, NH // 2, N], f32, tag="attnT")
                wp = ph2.tile([P, NH * D // P, C], f32, tag="wp")  # [128,3,768]
                for g3 in range(NH * D // P):
                    nc.sync.dma_start(wp[:, g3, :], wproj[g3 * P : (g3 + 1) * P, :])

                # ---- phase 2: attention per head ----------------------
                for h in range(NH):
                    g = h // 2
                    off = (h % 2) * D
                    av = psum.tile([D + 1, N], f32, tag="ps")
                    for kb in range(KB):
                        st = psum.tile([P, N], f32, tag="ps")
                        for qc in range(QC):
                            nc.tensor.matmul(
                                st[:, qc * 512 : (qc + 1) * 512],
                                qkT[off : off + D, 3 + g, kb * P : (kb + 1) * P],
                                qkT[off : off + D, g, qc * 512 : (qc + 1) * 512],
                                start=True,
                                stop=True,
                            )
                        nc.vector.tensor_mul(
                            st[:, kb * P : (kb + 1) * P],
                            st[:, kb * P : (kb + 1) * P],
                            masks[:, h, :],
                        )
                        pt = ptp.tile([P, N], f32, tag="pt")
                        nc.scalar.activation(pt[:], st[:], Exp, scale=SCALE)
                        for qc in range(QC):
                            nc.tensor.matmul(
                                av[:, qc * 512 : (qc + 1) * 512],
                                vaug[:, kb, h, :],
                                pt[:, qc * 512 : (qc + 1) * 512],
                                start=(kb == 0),
                                stop=(kb == KB - 1),
                            )
                    # normalize: rows 0..63 / row 64 (bcast via DRAM roundtrip)
                    rb = rbp.tile([P, N], f32, tag="rb")
                    nc.vector.reciprocal(rb[D : D + 1, :], av[D : D + 1, :])
                    nc.sync.dma_start(rdram[h, :], rb[D : D + 1, :])
                    nc.sync.dma_start(
                        rb[0:D, :], rdram[h : h + 1, :].broadcast_to([D, N])
                    )
                    nc.vector.tensor_mul(
                        attnT[off : off + D, g, :], av[0:D, :], rb[0:D, :]
                    )

                # ---- phase 3: output projection (transposed) ----------
                for m in range(CT):
                    po = psum.tile([P, N], f32, tag="ps")
                    for g3 in range(NH * D // P):
                        for qc in range(QC):
                            nc.tensor.matmul(
                                po[:, qc * 512 : (qc + 1) * 512],
                                wp[:, g3, m * P : (m + 1) * P],
                                attnT[:, g3, qc * 512 : (qc + 1) * 512],
                                start=(g3 == 0),
                                stop=(g3 == NH * D // P - 1),
                            )
                    ot = ptp.tile([P, N], f32, tag="pt")
                    nc.scalar.copy(ot[:], po[:])
                    nc.sync.dma_start(outT[m * P : (m + 1) * P, :], ot[:])

    if not nc.is_finalized():
        nc.finalize()
    return nc


def _get_program():
    if "nc" not in _CACHE:
        _CACHE["nc"] = _build_program()
    return _CACHE["nc"]


def _in_maps(x, w_qkv, w_proj, temperature):
    t = np.asarray(temperature, dtype=np.float32).reshape(H)
    maps = []
    xTs = {}
    for c in range(8):
        b, h0 = c // 2, NH * (c % 2)
        if b not in xTs:
            xTs[b] = np.ascontiguousarray(np.asarray(x[b], dtype=np.float32).T)
        cols = slice(D * h0, D * h0 + NH * D)
        wq = np.concatenate(
            [w_qkv[:, cols], w_qkv[:, C:][:, cols], w_qkv[:, 2 * C :][:, cols]],
            axis=1,
        )
        maps.append(
            {
                "xT": xTs[b],
                "wqkv": np.ascontiguousarray(wq, dtype=np.float32),
                "wproj": np.ascontiguousarray(
                    w_proj[D * h0 : D * h0 + NH * D, :], dtype=np.float32
                ),
                "temp": np.ascontiguousarray(
                    np.broadcast_to(t[h0 : h0 + NH].reshape(1, NH), (P, NH))
                ),
            }
        )
    return maps


def _install_profile_hook():
    """The agent image's antenv lacks axon_hooks; synthesize it and register
    the ctypes NTFF hook so run_bass_kernel_spmd(trace=True) can profile."""
    import types, importlib

    if "antenv.axon_hooks" not in sys.modules:
        import antenv

        mod = types.ModuleType("antenv.axon_hooks")
        _state = {"hook": None}
        mod.set_axon_ntff_profile_hook = lambda h: _state.__setitem__("hook", h)
        mod.get_axon_ntff_profile_hook = lambda: _state["hook"]
        sys.modules["antenv.axon_hooks"] = mod
        antenv.axon_hooks = mod
    from antenv.axon_hooks import (
        get_axon_ntff_profile_hook,
        set_axon_ntff_profile_hook,
    )

    if get_axon_ntff_profile_hook() is None:
        tb = importlib.import_module("trn_agent_boot.trn_boot")
        hook = tb._ntff_profile_via_ctypes("/opt/axon/libaxon_pjrt.so")
        set_axon_ntff_profile_hook(hook)


def kernel(x, w_qkv, w_proj, b_proj, temperature, _trace=False):
    from concourse.bass_utils import run_bass_kernel_spmd

    if _trace:
        try:
            _install_profile_hook()
        except Exception as e:  # profiling is best-effort
            print(f"profile hook install failed: {e}")

    nc = _get_program()
    maps = _in_maps(
        np.asarray(x, np.float32),
        np.asarray(w_qkv, np.float32),
        np.asarray(w_proj, np.float32),
        np.asarray(temperature, np.float32),
    )
    res = run_bass_kernel_spmd(nc, maps, list(range(8)), trace=_trace)
    parts = [r["outT"] for r in res.results]
    bp = np.asarray(b_proj, np.float32)
    out = np.stack(
        [(parts[2 * b] + parts[2 * b + 1]).T + bp for b in range(B)]
    ).astype(np.float32)
    if _trace:
        _CACHE["last_result"] = res
    return out
